# revision 117
# baseline (speedup 1.0000x reference)
"""Bayes predictor (retrieval-kNN softmax) Trainium2 kernel, 8 NeuronCores.

Math (reference):
    logits[b,n] = -(0.5*D*log(var_b) + 0.5/var_b * ||inputs_b - sqrt(a_b)*data_n||^2)
    probs = softmax(logits, axis=n);  x0 = probs @ data
    out = (inputs - sqrt(a)*x0)/sqrt(var)

Per-row-b softmax is invariant to per-b constants, so with
    s1_b = sqrt(a_b)/var_b,  s2_b = -a_b/(2 var_b),  r_n = ||data_n||^2
we use  l[b,n] = s1_b*(inputs_b . data_n) + s2_b*r_n.

Sharding: data_batch split along N across 8 cores (4096 rows each).
Each core computes a partial (max, weighted-sum, sum) triple; one
AllGather + a local combine produces the full output on every core.

Per-core layout ("packed-j"): logits live in PSUM as [128, 2, 512]
where partition p = 32*j + b covers n-group j (j in 0..3, 1024 n per
group), so vector/scalar ops run at full 128-partition width.

mm1 runs in float16 (1 cycle/row): a single pass per (half, j) chunk
with tile_position col-tiling (f16 error ~0.04 logits, well under the
tolerance), plus one stacked block-diagonal [12, 128] matmul per half
that adds s2_b*r_n (hi/lo split, 3 terms per j) to all 128 partitions.

The per-core softmax runs in the LOCAL per-partition frame (exp bias
is just the per-partition max, ready immediately), so exp starts right
after mm1. The j-group combine factors f_j(b) = exp(m_jb - M_core(b))
and the -M_core payload column are built off the critical path with
one PE transpose + j-min reduce + tiny spread matmuls. E^T tiles come
from paired PE transposes with a single 256-wide PSUM->SBUF copy per
pair; mm2 accumulates per-j [x0 | s] tiles (denominator via the ones
column of naug), and the j-combine is a 4-op scalar chain that
overlaps mm2's trailing matmuls.

The cross-core combine is a two-chain scalar_tensor_tensor
accumulation over the gathered payloads.
"""

import numpy as np

import concourse.bass as bass
import concourse.mybir as mybir
import concourse.tile as tile
from concourse import bacc
from concourse.bass_utils import run_bass_kernel_spmd

B, N, D = 32, 32768, 128
NCORES = 8
SHARD = N // NCORES          # 4096
NJ = 4                       # n-groups per core (partition-packed)
NG = SHARD // NJ             # 1024 n per group
NH = 2                       # halves (PSUM banks) per group row: 2*512
HW = 512                     # half width

F32 = mybir.dt.float32
F16 = mybir.dt.float16
BF16 = mybir.dt.bfloat16

# cbA [128, CA] f32
O_IDF = 0          # identity f32 [128, 128]
O_IDB = 128        # identity bf16 [128, 128] packed -> 64 words
CA = 192

# cbB [32, CB] f32
O_ISC = 0          # inputs/sqrt(var) [32, 128]
O_C2 = 128         # -sqrt(a)/sqrt(var) [32, 1]
O_ONES = 129       # ones [32, 128]
CB = 257

# cbC [12, CC] f16: 3 rows per j: (s2h, rh), (s2l, rh), (s2h, rl)
O_R = 0            # R12[3j+t, h, x]  [12, 2, 512]
O_SR = 1024        # Sr12[3j+t, 32j+b] (block diag) [12, 128]
O_ONE1 = 1152      # ones f16
O_ID2 = 1154       # 2x2 identity f16
CC = 1160

NQ = SHARD // 128  # 32 naug chunks
NW = 65            # f32 words per naug chunk row (130 bf16: 128 data, 1, pad)
FW = 130           # AllGather payload f32: [x0(128) | s | -M_core]

import os as _os
N_FILL = int(_os.environ.get("K_N_FILL", "0"))    # PE warm-up fillers
W_FILL = int(_os.environ.get("K_W_FILL", "32"))   # filler output free size

_CACHE = {}

_STAGES = ["mm1", "exp", "et", "mm2", "agr", "full"]


def _build(with_collective=True, stage="full"):
    sidx = _STAGES.index(stage)
    nc = bacc.Bacc("TRN2", target_bir_lowering=False, debug=False,
                   num_devices=NCORES)

    cbw_d = nc.dram_tensor("cbw", [128, B], F16, kind="ExternalInput")
    cba_d = nc.dram_tensor("cba", [128, CA], F32, kind="ExternalInput")
    cbb_d = nc.dram_tensor("cbb", [B, CB], F32, kind="ExternalInput")
    cbc_d = nc.dram_tensor("cbc", [12, CC], F16, kind="ExternalInput")
    dt_d = nc.dram_tensor("dt", [128, NH * NJ, HW], F16, kind="ExternalInput")
    naug_d = nc.dram_tensor("naug", [128, NQ, NW], F32, kind="ExternalInput")

    out_d = nc.dram_tensor("out", [B, D], F32, kind="ExternalOutput")

    ag_in = nc.dram_tensor("ag_in_b", [B, FW], F32)
    ag_out = nc.dram_tensor("ag_out_b", [B * NCORES, FW], F32,
                            addr_space="Shared")

    with tile.TileContext(nc) as tc:
        with (
            tc.tile_pool(name="sb", bufs=1) as sb,
            tc.tile_pool(name="ps_l", bufs=1, space="PSUM") as ps_l,
            tc.tile_pool(name="ps_et", bufs=2, space="PSUM") as ps_et,
            tc.tile_pool(name="ps_x", bufs=1, space="PSUM") as ps_x,
            tc.tile_pool(name="ps_a", bufs=1, space="PSUM") as ps_a,
        ):
            # ---- activation-table warm-up (pulls Exp table load to t=0) --
            warm = sb.tile([1, 2], F32)
            nc.vector.memset(warm[:, 0:1], 0.0)
            nc.scalar.activation(warm[:, 1:2], warm[:, 0:1],
                                 mybir.ActivationFunctionType.Exp,
                                 bias=warm[:, 0:1])

            # ---- input DMAs (issue order = transfer order) ----
            w16 = sb.tile([128, B], F16)
            nc.sync.dma_start(out=w16, in_=cbw_d.ap())
            dt = sb.tile([128, NH * NJ, HW], F16)
            cbc = sb.tile([12, CC], F16)
            for h2 in range(4):
                nc.sync.dma_start(out=dt[:, 2 * h2:2 * h2 + 2, :],
                                  in_=dt_d.ap()[:, 2 * h2:2 * h2 + 2, :])
                if h2 == 0:
                    nc.sync.dma_start(out=cbc, in_=cbc_d.ap())
            cba = sb.tile([128, CA], F32)
            nc.sync.dma_start(out=cba, in_=cba_d.ap())
            cbb = sb.tile([B, CB], F32)
            nc.sync.dma_start(out=cbb, in_=cbb_d.ap())
            naug = sb.tile([128, NQ, NW], F32)
            for q in range(2):
                nc.sync.dma_start(
                    out=naug[:, q * (NQ // 2):(q + 1) * (NQ // 2), :],
                    in_=naug_d.ap()[:, q * (NQ // 2):(q + 1) * (NQ // 2), :],
                )
            nbf = naug.bitcast(BF16)       # [128, NQ, 2*NW]

            def dchunk(q):
                return dt[:, q, :]

            identf = cba[:, O_IDF:O_IDF + 128]
            identb = cba[:, O_IDB:O_IDB + 64].bitcast(BF16)
            inputs_sc = cbb[:, O_ISC:O_ISC + D]
            c2neg = cbb[:, O_C2:O_C2 + 1]
            ones_row = cbb[0:1, O_ONES:O_ONES + 128]         # [1, 128]
            one11 = cbb[0:1, O_ONES:O_ONES + 1]              # [1, 1]
            r12 = cbc[:, O_R:O_R + NH * HW]                  # [12, 1024]
            sr12 = cbc[:, O_SR:O_SR + 128]                   # [12, 128]
            one16 = cbc[0:1, O_ONE1:O_ONE1 + 1]              # [1, 1] f16
            ident2 = cbc[0:2, O_ID2:O_ID2 + 2]               # [2, 2] f16

            # shared 1-bank PSUM scratch: fillers, nm1 transpose, f_row
            # broadcast, and the [32,1] -M spread (writes are ordered)
            aux_ps = ps_a.tile([128, 128], F32)

            # ---- PE p-state warm-up fillers (tiny matmuls on W) ----
            for k in range(N_FILL):
                nc.tensor.matmul(aux_ps[0:B, 0:W_FILL], w16,
                                 w16[:, 0:W_FILL], start=True, stop=True)

            # ---- mm1: logits into two single-bank PSUM tiles (separate
            # tiles so each bank's readers release independently) ----
            l_b = []
            for h in range(NH):
                l_h = ps_l.tile([128, HW], F32, tag=f"l{h}")
                l_b.append(l_h)
                for j in range(NJ):
                    nc.tensor.matmul(
                        l_h[32 * j:32 * j + 32, :], w16, dchunk(4 * h + j),
                        start=True, stop=False, tile_position=(0, 32 * j),
                    )
                # adds s2_b * r_n on all 128 partitions in one matmul
                nc.tensor.matmul(
                    l_h, sr12, r12[:, h * HW:(h + 1) * HW],
                    start=False, stop=True, tile_position=(0, 0),
                )

            if sidx >= 1:
                # ---- per-partition -max; exp in the LOCAL frame so it
                # starts as soon as nm1 merges (no cross-partition wait) ---
                nmh = sb.tile([128, NH], F32)
                for h in range(NH):
                    nc.vector.tensor_reduce(nmh[:, h:h + 1], l_b[h],
                                            axis=mybir.AxisListType.X,
                                            op=mybir.AluOpType.max,
                                            negate=True)
                nm1 = sb.tile([128, 1], F32)
                nc.vector.tensor_reduce(nm1, nmh, axis=mybir.AxisListType.X,
                                        op=mybir.AluOpType.min)
                e_sb = sb.tile([128, NG], BF16)
                for h in range(NH):
                    nc.scalar.activation(e_sb[:, HW * h:HW * h + HW], l_b[h],
                                         mybir.ActivationFunctionType.Exp,
                                         bias=nm1)

                # ---- off-critical: f_j[b] = exp(m_p - M_core(b)) per
                # j-group as a [32, 4] tile + the -M payload column, via
                # one transpose, j-min, and tiny spread matmuls ----
                nc.tensor.transpose(aux_ps[0:1, :], nm1, identf)
                nmc = sb.tile([1, B], F16)     # -M_core per b (f16; same
                nc.vector.tensor_reduce(      # values used everywhere)
                    nmc,
                    aux_ps[0:1, :].rearrange("p (j b) -> p b j", j=NJ),
                    axis=mybir.AxisListType.X, op=mybir.AluOpType.min)
                drow = sb.tile([1, 128], F16)  # m_p - M_core(b(p))
                nc.vector.tensor_tensor(
                    drow.rearrange("p (j b) -> p j b", j=NJ),
                    nmc[:, None, :].broadcast_to([1, NJ, B]),
                    aux_ps[0:1, :].rearrange("p (j b) -> p j b", j=NJ),
                    op=mybir.AluOpType.subtract)
                frow = sb.tile([1, 128], F16)
                nc.scalar.activation(frow, drow,
                                     mybir.ActivationFunctionType.Exp)
                # aux cols 1..4 <- f_j columns, col 5 <- -M_core
                for j in range(NJ):
                    nc.tensor.matmul(aux_ps[0:B, 1 + j:2 + j],
                                     frow[:, 32 * j:32 * j + 32], one16,
                                     start=True, stop=True)
                nc.tensor.matmul(aux_ps[0:B, 5:6], nmc, one16,
                                 start=True, stop=True)
                fqm = sb.tile([B, 5], F32)     # [f_0..f_3 | -M]
                nc.vector.tensor_copy(fqm, aux_ps[0:B, 1:6])

            if sidx >= 2:
                # ---- E^T tiles: transpose pairs into one PSUM tile, then
                # one 256-wide copy per pair ----
                et_sb = sb.tile([128, NG // 128, 128], BF16)
                for cp in range(NG // 256):
                    etp = ps_et.tile([128, 256], BF16, tag="et")
                    for i in range(2):
                        c = 2 * cp + i
                        nc.tensor.transpose(
                            etp[:, 128 * i:128 * i + 128],
                            e_sb[:, 128 * c:128 * c + 128], identb)
                    nc.vector.tensor_copy(
                        et_sb.rearrange("p c w -> p (c w)")
                        [:, 256 * cp:256 * cp + 256], etp)

            if sidx >= 3:
                # ---- mm2: per-j [x0 | s] tiles (two per PSUM bank); the
                # j-combine overlaps the later j-groups' matmuls ----
                xt0 = ps_x.tile([B, D + 1], F32, tag="x0")
                xt1 = ps_x.tile([B, D + 1], F32, tag="x1")
                xt2 = ps_x.tile([B, 2, D + 1], F32, tag="x2")
                x_j = [xt0, xt1, xt2[:, 0, :], xt2[:, 1, :]]
                for j in range(NJ):
                    for c in range(NG // 128):
                        nc.tensor.matmul(
                            x_j[j], et_sb[:, c, 32 * j:32 * j + 32],
                            nbf[:, (NG // 128) * j + c, 0:D + 1],
                            start=(c == 0), stop=(c == NG // 128 - 1),
                        )

                # ---- AllGather payload [x0 | s | -M], combined in SBUF ---
                agi = sb.tile([B, FW], F32)
                nc.vector.tensor_copy(agi[:, D + 1:D + 2], fqm[:, 4:5])
                acc = agi[:, 0:D + 1]
                nc.vector.tensor_scalar_mul(acc, x_j[0], fqm[:, 0:1])
                for j in range(1, NJ):
                    nc.vector.scalar_tensor_tensor(
                        acc, x_j[j], fqm[:, j:j + 1], acc,
                        op0=mybir.AluOpType.mult, op1=mybir.AluOpType.add,
                    )
                nc.sync.dma_start(out=ag_in.ap(), in_=agi)

            if sidx >= 4:
                agg = sb.tile([B, NCORES, FW], F32)
                if with_collective:
                    nc.gpsimd.collective_compute(
                        "AllGather",
                        mybir.AluOpType.bypass,
                        replica_groups=[list(range(NCORES))],
                        ins=[ag_in.ap().opt()],
                        outs=[ag_out.ap().opt()],
                    )
                    nc.sync.dma_start(
                        out=agg,
                        in_=ag_out.ap().rearrange("(c p) f -> p c f", p=B),
                    )
                else:
                    # timing-sim stand-in (collective itself not modeled):
                    # same payload store + a broadcast read-back
                    nc.sync.dma_start(
                        out=agg,
                        in_=ag_in.ap()[:, None, :].broadcast_to(
                            [B, NCORES, FW]),
                    )

            if sidx >= 5:
                # ---- cross-core combine (identical on every core) ----
                nmg8 = agg[:, :, D + 1]          # [32, 8] strided view
                nmming = sb.tile([B, 1], F32)    # = -M_global
                nc.vector.tensor_reduce(nmming, nmg8,
                                        axis=mybir.AxisListType.X,
                                        op=mybir.AluOpType.min)
                fg = sb.tile([B, NCORES], F32)   # exp(M_c - M_global)
                nc.scalar.activation(fg, nmg8,
                                     mybir.ActivationFunctionType.Exp,
                                     bias=nmming, scale=-1.0)
                # two independent accumulator chains, interleaved on DVE
                acc2 = sb.tile([B, 2, D + 1], F32)
                for p in range(2):
                    nc.vector.tensor_scalar_mul(acc2[:, p, :],
                                                agg[:, p, 0:D + 1],
                                                fg[:, p:p + 1])
                for c in range(2, NCORES):
                    nc.vector.scalar_tensor_tensor(
                        acc2[:, c % 2, :], agg[:, c, 0:D + 1],
                        fg[:, c:c + 1], acc2[:, c % 2, :],
                        op0=mybir.AluOpType.mult, op1=mybir.AluOpType.add,
                    )
                accg = sb.tile([B, D + 1], F32)
                nc.vector.tensor_tensor(accg, acc2[:, 0, :], acc2[:, 1, :],
                                        op=mybir.AluOpType.add)

                # ---- final: out = x0_tot * (c2neg/s_tot) + inputs_sc ----
                rec = sb.tile([B, 1], F32)
                nc.vector.reciprocal(rec, accg[:, D:D + 1])
                c2r = sb.tile([B, 1], F32)
                nc.vector.tensor_tensor(c2r, rec, c2neg,
                                        op=mybir.AluOpType.mult)
                outt = sb.tile([B, D], F32)
                nc.vector.scalar_tensor_tensor(
                    outt, accg[:, 0:D], c2r, inputs_sc,
                    op0=mybir.AluOpType.mult, op1=mybir.AluOpType.add,
                )
                nc.sync.dma_start(out=out_d.ap(), in_=outt)

    nc.compile()
    return nc


def _get_nc():
    if "nc" not in _CACHE:
        _CACHE["nc"] = _build()
    return _CACHE["nc"]


def _prepare_in_maps(inputs, alphas, data_batch):
    import ml_dtypes

    inputs = np.asarray(inputs, np.float32)
    alphas = np.asarray(alphas, np.float32)
    data = np.ascontiguousarray(np.asarray(data_batch, np.float32))

    var = 1.0 - alphas
    s1 = np.sqrt(alphas) / var                        # [B]
    s2 = -alphas / (2.0 * var)                        # [B]
    w_all = (inputs * s1[:, None]).T.astype(np.float16)   # [D, B] f16
    inputs_sc = (inputs / np.sqrt(var)[:, None]).astype(np.float32)
    c2neg = (-np.sqrt(alphas) / np.sqrt(var)).astype(np.float32)

    dataT = np.ascontiguousarray(data.T)              # [D, N]
    r = (data * data).sum(axis=1).astype(np.float32)  # [N]
    r_h = r.astype(np.float16)
    r_l = (r - r_h.astype(np.float32)).astype(np.float16)
    s2_h = s2.astype(np.float16)
    s2_l = (s2 - s2_h.astype(np.float32)).astype(np.float16)

    identf = np.eye(128, dtype=np.float32)
    identb = np.eye(128, dtype=ml_dtypes.bfloat16)
    identb_w = np.ascontiguousarray(identb).view(np.uint16).view(np.float32)

    cba = np.zeros((128, CA), np.float32)
    cba[:, O_IDF:O_IDF + 128] = identf
    cba[:, O_IDB:O_IDB + 64] = identb_w

    cbb = np.zeros((B, CB), np.float32)
    cbb[:, O_ISC:O_ISC + D] = inputs_sc
    cbb[:, O_C2] = c2neg
    cbb[:, O_ONES:O_ONES + 128] = 1.0

    # Sr12: block-diag rows (s2h, s2l, s2h) per j
    sr12 = np.zeros((12, 128), np.float16)
    for j in range(NJ):
        sr12[3 * j + 0, 32 * j:32 * j + 32] = s2_h
        sr12[3 * j + 1, 32 * j:32 * j + 32] = s2_l
        sr12[3 * j + 2, 32 * j:32 * j + 32] = s2_h

    in_maps = []
    for cid in range(NCORES):
        lo = cid * SHARD
        dt_c = dataT[:, lo:lo + SHARD].astype(np.float16)  # [128, 4096]

        # chunk (h, j) = dataT cols 1024j + 512h + x
        dtt = np.empty((128, NH * NJ, HW), np.float16)
        for h in range(NH):
            for j in range(NJ):
                dtt[:, 4 * h + j, :] = dt_c[:, 1024 * j + 512 * h:
                                            1024 * j + 512 * h + HW]

        # R12 rows per j: (rh, rh, rl)
        cbc = np.zeros((12, CC), np.float16)
        for j in range(NJ):
            for h in range(NH):
                sl = slice(lo + 1024 * j + 512 * h,
                           lo + 1024 * j + 512 * h + HW)
                cbc[3 * j + 0, O_R + h * HW:O_R + (h + 1) * HW] = r_h[sl]
                cbc[3 * j + 1, O_R + h * HW:O_R + (h + 1) * HW] = r_h[sl]
                cbc[3 * j + 2, O_R + h * HW:O_R + (h + 1) * HW] = r_l[sl]
        cbc[:, O_SR:O_SR + 128] = sr12
        cbc[:, O_ONE1] = 1.0
        cbc[0, O_ID2] = 1.0
        cbc[1, O_ID2 + 1] = 1.0

        # naug chunks: [128 rows, 130 bf16] = [data | 1.0 | 0]
        nrows = np.zeros((SHARD, 2 * NW), ml_dtypes.bfloat16)
        nrows[:, 0:D] = data[lo:lo + SHARD].astype(ml_dtypes.bfloat16)
        nrows[:, D] = 1.0
        naug = np.ascontiguousarray(
            nrows.reshape(NQ, 128, 2 * NW).transpose(1, 0, 2)
        ).view(np.uint16).view(np.float32)            # [128, NQ, NW]

        in_maps.append({
            "cbw": w_all,
            "cba": cba,
            "cbb": cbb,
            "cbc": cbc,
            "dt": dtt,
            "naug": naug,
        })
    return in_maps


def run(inputs, alphas, data_batch, trace=False, trace_kwargs=None):
    nc = _get_nc()
    in_maps = _prepare_in_maps(inputs, alphas, data_batch)
    res = run_bass_kernel_spmd(
        nc, in_maps, core_ids=list(range(NCORES)),
        trace=trace, **(trace_kwargs or {}),
    )
    return res.results[0]["out"].astype(np.float32), res


def kernel(inputs, alphas, data_batch):
    out, _ = run(inputs, alphas, data_batch)
    return out


# revision 123
# speedup vs baseline: 1.0054x; 1.0054x over previous
"""Bayes predictor (retrieval-kNN softmax) Trainium2 kernel, 8 NeuronCores.

Math (reference):
    logits[b,n] = -(0.5*D*log(var_b) + 0.5/var_b * ||inputs_b - sqrt(a_b)*data_n||^2)
    probs = softmax(logits, axis=n);  x0 = probs @ data
    out = (inputs - sqrt(a)*x0)/sqrt(var)

Per-row-b softmax is invariant to per-b constants, so with
    s1_b = sqrt(a_b)/var_b,  s2_b = -a_b/(2 var_b),  r_n = ||data_n||^2
we use  l[b,n] = s1_b*(inputs_b . data_n) + s2_b*r_n.

Sharding: data_batch split along N across 8 cores (4096 rows each).
Each core computes a partial (max, weighted-sum, sum) triple; one
AllGather + a local combine produces the full output on every core.

Per-core layout ("packed-j"): logits live in PSUM as [128, 2, 512]
where partition p = 32*j + b covers n-group j (j in 0..3, 1024 n per
group), so vector/scalar ops run at full 128-partition width.

mm1 runs in float16 (1 cycle/row): a single pass per (half, j) chunk
with tile_position col-tiling (f16 error ~0.04 logits, well under the
tolerance), plus one stacked block-diagonal [12, 128] matmul per half
that adds s2_b*r_n (hi/lo split, 3 terms per j) to all 128 partitions.

The per-core softmax runs in the LOCAL per-partition frame (exp bias
is just the per-partition max, ready immediately), so exp starts right
after mm1. The j-group combine factors f_j(b) = exp(m_jb - M_core(b))
and the -M_core payload column are built off the critical path with
one PE transpose + j-min reduce + tiny spread matmuls. E^T tiles come
from paired PE transposes with a single 256-wide PSUM->SBUF copy per
pair; mm2 accumulates per-j [x0 | s] tiles (denominator via the ones
column of naug), and the j-combine is a 4-op scalar chain that
overlaps mm2's trailing matmuls.

The cross-core combine is a two-chain scalar_tensor_tensor
accumulation over the gathered payloads.
"""

import numpy as np

import concourse.bass as bass
import concourse.mybir as mybir
import concourse.tile as tile
from concourse import bacc
from concourse.bass_utils import run_bass_kernel_spmd

B, N, D = 32, 32768, 128
NCORES = 8
SHARD = N // NCORES          # 4096
NJ = 4                       # n-groups per core (partition-packed)
NG = SHARD // NJ             # 1024 n per group
NH = 2                       # halves (PSUM banks) per group row: 2*512
HW = 512                     # half width

F32 = mybir.dt.float32
F16 = mybir.dt.float16
BF16 = mybir.dt.bfloat16

# cbA [128, CA] f32
O_IDF = 0          # identity f32 [128, 128]
O_IDB = 128        # identity bf16 [128, 128] packed -> 64 words
CA = 192

# cbB [32, CB] f32
O_ISC = 0          # inputs/sqrt(var) [32, 128]
O_C2 = 128         # -sqrt(a)/sqrt(var) [32, 1]
O_ONES = 129       # ones [32, 128]
CB = 257

# cbC [12, CC] f16: 3 rows per j: (s2h, rh), (s2l, rh), (s2h, rl)
O_R = 0            # R12[3j+t, h, x]  [12, 2, 512]
O_SR = 1024        # Sr12[3j+t, 32j+b] (block diag) [12, 128]
O_ONE1 = 1152      # ones f16
O_ID2 = 1154       # 2x2 identity f16
CC = 1160

NQ = SHARD // 128  # 32 naug chunks
NW = 65            # f32 words per naug chunk row (130 bf16: 128 data, 1, pad)
FW = 130           # AllGather payload f32: [x0(128) | s | -M_core]

import os as _os
N_FILL = int(_os.environ.get("K_N_FILL", "0"))    # PE warm-up fillers
W_FILL = int(_os.environ.get("K_W_FILL", "32"))   # filler output free size

_CACHE = {}

_STAGES = ["mm1", "exp", "et", "mm2", "agr", "full"]


def _build(with_collective=True, stage="full"):
    sidx = _STAGES.index(stage)
    nc = bacc.Bacc("TRN2", target_bir_lowering=False, debug=False,
                   num_devices=NCORES)

    cbw_d = nc.dram_tensor("cbw", [128, B], F16, kind="ExternalInput")
    cba_d = nc.dram_tensor("cba", [128, CA], F32, kind="ExternalInput")
    cbb_d = nc.dram_tensor("cbb", [B, CB], F32, kind="ExternalInput")
    cbc_d = nc.dram_tensor("cbc", [12, CC], F16, kind="ExternalInput")
    dt_d = nc.dram_tensor("dt", [128, NH * NJ, HW], F16, kind="ExternalInput")
    naug_d = nc.dram_tensor("naug", [128, NQ, NW], F32, kind="ExternalInput")

    out_d = nc.dram_tensor("out", [B, D], F32, kind="ExternalOutput")

    ag_in = nc.dram_tensor("ag_in_b", [B, FW], F32)
    ag_out = nc.dram_tensor("ag_out_b", [B * NCORES, FW], F32,
                            addr_space="Shared")

    with tile.TileContext(nc) as tc:
        with (
            tc.tile_pool(name="sb", bufs=1) as sb,
            tc.tile_pool(name="ps_l", bufs=1, space="PSUM") as ps_l,
            tc.tile_pool(name="ps_et", bufs=2, space="PSUM") as ps_et,
            tc.tile_pool(name="ps_x", bufs=1, space="PSUM") as ps_x,
            tc.tile_pool(name="ps_a", bufs=1, space="PSUM") as ps_a,
        ):
            # ---- activation-table warm-up (pulls Exp table load to t=0) --
            warm = sb.tile([1, 2], F32)
            nc.vector.memset(warm[:, 0:1], 0.0)
            nc.scalar.activation(warm[:, 1:2], warm[:, 0:1],
                                 mybir.ActivationFunctionType.Exp,
                                 bias=warm[:, 0:1])

            # ---- input DMAs (issue order = transfer order) ----
            w16 = sb.tile([128, B], F16)
            nc.sync.dma_start(out=w16, in_=cbw_d.ap())
            dt = sb.tile([128, NH * NJ, HW], F16)
            cbc = sb.tile([12, CC], F16)
            for h2 in range(4):
                nc.sync.dma_start(out=dt[:, 2 * h2:2 * h2 + 2, :],
                                  in_=dt_d.ap()[:, 2 * h2:2 * h2 + 2, :])
                if h2 == 2:
                    nc.sync.dma_start(out=cbc, in_=cbc_d.ap())
            cba = sb.tile([128, CA], F32)
            nc.sync.dma_start(out=cba, in_=cba_d.ap())
            cbb = sb.tile([B, CB], F32)
            nc.sync.dma_start(out=cbb, in_=cbb_d.ap())
            naug = sb.tile([128, NQ, NW], F32)
            for q in range(2):
                nc.sync.dma_start(
                    out=naug[:, q * (NQ // 2):(q + 1) * (NQ // 2), :],
                    in_=naug_d.ap()[:, q * (NQ // 2):(q + 1) * (NQ // 2), :],
                )
            nbf = naug.bitcast(BF16)       # [128, NQ, 2*NW]

            def dchunk(q):
                return dt[:, q, :]

            identf = cba[:, O_IDF:O_IDF + 128]
            identb = cba[:, O_IDB:O_IDB + 64].bitcast(BF16)
            inputs_sc = cbb[:, O_ISC:O_ISC + D]
            c2neg = cbb[:, O_C2:O_C2 + 1]
            ones_row = cbb[0:1, O_ONES:O_ONES + 128]         # [1, 128]
            one11 = cbb[0:1, O_ONES:O_ONES + 1]              # [1, 1]
            r12 = cbc[:, O_R:O_R + NH * HW]                  # [12, 1024]
            sr12 = cbc[:, O_SR:O_SR + 128]                   # [12, 128]
            one16 = cbc[0:1, O_ONE1:O_ONE1 + 1]              # [1, 1] f16
            ident2 = cbc[0:2, O_ID2:O_ID2 + 2]               # [2, 2] f16

            # shared 1-bank PSUM scratch: fillers, nm1 transpose, f_row
            # broadcast, and the [32,1] -M spread (writes are ordered)
            aux_ps = ps_a.tile([128, 128], F32)

            # ---- PE p-state warm-up fillers (tiny matmuls on W) ----
            for k in range(N_FILL):
                nc.tensor.matmul(aux_ps[0:B, 0:W_FILL], w16,
                                 w16[:, 0:W_FILL], start=True, stop=True)

            # ---- mm1: logits into two single-bank PSUM tiles (separate
            # tiles so each bank's readers release independently) ----
            l_b = []
            for h in range(NH):
                l_h = ps_l.tile([128, HW], F32, tag=f"l{h}")
                l_b.append(l_h)
                for j in range(NJ):
                    nc.tensor.matmul(
                        l_h[32 * j:32 * j + 32, :], w16, dchunk(4 * h + j),
                        start=True, stop=False, tile_position=(0, 32 * j),
                    )
                # adds s2_b * r_n on all 128 partitions in one matmul
                nc.tensor.matmul(
                    l_h, sr12, r12[:, h * HW:(h + 1) * HW],
                    start=False, stop=True, tile_position=(0, 0),
                )

            if sidx >= 1:
                # ---- per-partition -max; exp in the LOCAL frame so it
                # starts as soon as nm1 merges (no cross-partition wait) ---
                nmh = sb.tile([128, NH], F32)
                for h in range(NH):
                    nc.vector.tensor_reduce(nmh[:, h:h + 1], l_b[h],
                                            axis=mybir.AxisListType.X,
                                            op=mybir.AluOpType.max,
                                            negate=True)
                nm1 = sb.tile([128, 1], F32)
                nc.vector.tensor_reduce(nm1, nmh, axis=mybir.AxisListType.X,
                                        op=mybir.AluOpType.min)
                e_sb = sb.tile([128, NG], BF16)
                for h in range(NH):
                    nc.scalar.activation(e_sb[:, HW * h:HW * h + HW], l_b[h],
                                         mybir.ActivationFunctionType.Exp,
                                         bias=nm1)

                # ---- off-critical: f_j[b] = exp(m_p - M_core(b)) per
                # j-group as a [32, 4] tile + the -M payload column, via
                # one transpose, j-min, and tiny spread matmuls ----
                nc.tensor.transpose(aux_ps[0:1, :], nm1, identf)
                nmc = sb.tile([1, B], F16)     # -M_core per b (f16; same
                nc.vector.tensor_reduce(      # values used everywhere)
                    nmc,
                    aux_ps[0:1, :].rearrange("p (j b) -> p b j", j=NJ),
                    axis=mybir.AxisListType.X, op=mybir.AluOpType.min)
                drow = sb.tile([1, 128], F16)  # m_p - M_core(b(p))
                nc.vector.tensor_tensor(
                    drow.rearrange("p (j b) -> p j b", j=NJ),
                    nmc[:, None, :].broadcast_to([1, NJ, B]),
                    aux_ps[0:1, :].rearrange("p (j b) -> p j b", j=NJ),
                    op=mybir.AluOpType.subtract)
                frow = sb.tile([1, 128], F16)
                nc.scalar.activation(frow, drow,
                                     mybir.ActivationFunctionType.Exp)
                # aux cols 1..4 <- f_j columns, col 5 <- -M_core
                for j in range(NJ):
                    nc.tensor.matmul(aux_ps[0:B, 1 + j:2 + j],
                                     frow[:, 32 * j:32 * j + 32], one16,
                                     start=True, stop=True)
                nc.tensor.matmul(aux_ps[0:B, 5:6], nmc, one16,
                                 start=True, stop=True)
                fqm = sb.tile([B, 5], F32)     # [f_0..f_3 | -M]
                nc.vector.tensor_copy(fqm, aux_ps[0:B, 1:6])

            if sidx >= 2:
                # ---- E^T tiles: transpose pairs into one PSUM tile, then
                # one 256-wide copy per pair ----
                et_sb = sb.tile([128, NG // 128, 128], BF16)
                for cp in range(NG // 256):
                    etp = ps_et.tile([128, 256], BF16, tag="et")
                    for i in range(2):
                        c = 2 * cp + i
                        nc.tensor.transpose(
                            etp[:, 128 * i:128 * i + 128],
                            e_sb[:, 128 * c:128 * c + 128], identb)
                    nc.vector.tensor_copy(
                        et_sb.rearrange("p c w -> p (c w)")
                        [:, 256 * cp:256 * cp + 256], etp)

            if sidx >= 3:
                # ---- mm2: per-j [x0 | s] tiles (two per PSUM bank); the
                # j-combine overlaps the later j-groups' matmuls ----
                xt0 = ps_x.tile([B, D + 1], F32, tag="x0")
                xt1 = ps_x.tile([B, D + 1], F32, tag="x1")
                xt2 = ps_x.tile([B, 2, D + 1], F32, tag="x2")
                x_j = [xt0, xt1, xt2[:, 0, :], xt2[:, 1, :]]
                for j in range(NJ):
                    for c in range(NG // 128):
                        nc.tensor.matmul(
                            x_j[j], et_sb[:, c, 32 * j:32 * j + 32],
                            nbf[:, (NG // 128) * j + c, 0:D + 1],
                            start=(c == 0), stop=(c == NG // 128 - 1),
                        )

                # ---- AllGather payload [x0 | s | -M], combined in SBUF ---
                agi = sb.tile([B, FW], F32)
                nc.vector.tensor_copy(agi[:, D + 1:D + 2], fqm[:, 4:5])
                acc = agi[:, 0:D + 1]
                nc.vector.tensor_scalar_mul(acc, x_j[0], fqm[:, 0:1])
                for j in range(1, NJ):
                    nc.vector.scalar_tensor_tensor(
                        acc, x_j[j], fqm[:, j:j + 1], acc,
                        op0=mybir.AluOpType.mult, op1=mybir.AluOpType.add,
                    )
                nc.sync.dma_start(out=ag_in.ap(), in_=agi)

            if sidx >= 4:
                agg = sb.tile([B, NCORES, FW], F32)
                if with_collective:
                    nc.gpsimd.collective_compute(
                        "AllGather",
                        mybir.AluOpType.bypass,
                        replica_groups=[list(range(NCORES))],
                        ins=[ag_in.ap().opt()],
                        outs=[ag_out.ap().opt()],
                    )
                    nc.sync.dma_start(
                        out=agg,
                        in_=ag_out.ap().rearrange("(c p) f -> p c f", p=B),
                    )
                else:
                    # timing-sim stand-in (collective itself not modeled):
                    # same payload store + a broadcast read-back
                    nc.sync.dma_start(
                        out=agg,
                        in_=ag_in.ap()[:, None, :].broadcast_to(
                            [B, NCORES, FW]),
                    )

            if sidx >= 5:
                # ---- cross-core combine (identical on every core) ----
                nmg8 = agg[:, :, D + 1]          # [32, 8] strided view
                nmming = sb.tile([B, 1], F32)    # = -M_global
                nc.vector.tensor_reduce(nmming, nmg8,
                                        axis=mybir.AxisListType.X,
                                        op=mybir.AluOpType.min)
                fg = sb.tile([B, NCORES], F32)   # exp(M_c - M_global)
                nc.scalar.activation(fg, nmg8,
                                     mybir.ActivationFunctionType.Exp,
                                     bias=nmming, scale=-1.0)
                # two independent accumulator chains, interleaved on DVE
                acc2 = sb.tile([B, 2, D + 1], F32)
                for p in range(2):
                    nc.vector.tensor_scalar_mul(acc2[:, p, :],
                                                agg[:, p, 0:D + 1],
                                                fg[:, p:p + 1])
                for c in range(2, NCORES):
                    nc.vector.scalar_tensor_tensor(
                        acc2[:, c % 2, :], agg[:, c, 0:D + 1],
                        fg[:, c:c + 1], acc2[:, c % 2, :],
                        op0=mybir.AluOpType.mult, op1=mybir.AluOpType.add,
                    )
                accg = sb.tile([B, D + 1], F32)
                nc.vector.tensor_tensor(accg, acc2[:, 0, :], acc2[:, 1, :],
                                        op=mybir.AluOpType.add)

                # ---- final: out = x0_tot * (c2neg/s_tot) + inputs_sc ----
                rec = sb.tile([B, 1], F32)
                nc.vector.reciprocal(rec, accg[:, D:D + 1])
                c2r = sb.tile([B, 1], F32)
                nc.vector.tensor_tensor(c2r, rec, c2neg,
                                        op=mybir.AluOpType.mult)
                outt = sb.tile([B, D], F32)
                nc.vector.scalar_tensor_tensor(
                    outt, accg[:, 0:D], c2r, inputs_sc,
                    op0=mybir.AluOpType.mult, op1=mybir.AluOpType.add,
                )
                nc.sync.dma_start(out=out_d.ap(), in_=outt)

    nc.compile()
    return nc


def _get_nc():
    if "nc" not in _CACHE:
        _CACHE["nc"] = _build()
    return _CACHE["nc"]


def _prepare_in_maps(inputs, alphas, data_batch):
    import ml_dtypes

    inputs = np.asarray(inputs, np.float32)
    alphas = np.asarray(alphas, np.float32)
    data = np.ascontiguousarray(np.asarray(data_batch, np.float32))

    var = 1.0 - alphas
    s1 = np.sqrt(alphas) / var                        # [B]
    s2 = -alphas / (2.0 * var)                        # [B]
    w_all = (inputs * s1[:, None]).T.astype(np.float16)   # [D, B] f16
    inputs_sc = (inputs / np.sqrt(var)[:, None]).astype(np.float32)
    c2neg = (-np.sqrt(alphas) / np.sqrt(var)).astype(np.float32)

    dataT = np.ascontiguousarray(data.T)              # [D, N]
    r = (data * data).sum(axis=1).astype(np.float32)  # [N]
    r_h = r.astype(np.float16)
    r_l = (r - r_h.astype(np.float32)).astype(np.float16)
    s2_h = s2.astype(np.float16)
    s2_l = (s2 - s2_h.astype(np.float32)).astype(np.float16)

    identf = np.eye(128, dtype=np.float32)
    identb = np.eye(128, dtype=ml_dtypes.bfloat16)
    identb_w = np.ascontiguousarray(identb).view(np.uint16).view(np.float32)

    cba = np.zeros((128, CA), np.float32)
    cba[:, O_IDF:O_IDF + 128] = identf
    cba[:, O_IDB:O_IDB + 64] = identb_w

    cbb = np.zeros((B, CB), np.float32)
    cbb[:, O_ISC:O_ISC + D] = inputs_sc
    cbb[:, O_C2] = c2neg
    cbb[:, O_ONES:O_ONES + 128] = 1.0

    # Sr12: block-diag rows (s2h, s2l, s2h) per j
    sr12 = np.zeros((12, 128), np.float16)
    for j in range(NJ):
        sr12[3 * j + 0, 32 * j:32 * j + 32] = s2_h
        sr12[3 * j + 1, 32 * j:32 * j + 32] = s2_l
        sr12[3 * j + 2, 32 * j:32 * j + 32] = s2_h

    in_maps = []
    for cid in range(NCORES):
        lo = cid * SHARD
        dt_c = dataT[:, lo:lo + SHARD].astype(np.float16)  # [128, 4096]

        # chunk (h, j) = dataT cols 1024j + 512h + x
        dtt = np.empty((128, NH * NJ, HW), np.float16)
        for h in range(NH):
            for j in range(NJ):
                dtt[:, 4 * h + j, :] = dt_c[:, 1024 * j + 512 * h:
                                            1024 * j + 512 * h + HW]

        # R12 rows per j: (rh, rh, rl)
        cbc = np.zeros((12, CC), np.float16)
        for j in range(NJ):
            for h in range(NH):
                sl = slice(lo + 1024 * j + 512 * h,
                           lo + 1024 * j + 512 * h + HW)
                cbc[3 * j + 0, O_R + h * HW:O_R + (h + 1) * HW] = r_h[sl]
                cbc[3 * j + 1, O_R + h * HW:O_R + (h + 1) * HW] = r_h[sl]
                cbc[3 * j + 2, O_R + h * HW:O_R + (h + 1) * HW] = r_l[sl]
        cbc[:, O_SR:O_SR + 128] = sr12
        cbc[:, O_ONE1] = 1.0
        cbc[0, O_ID2] = 1.0
        cbc[1, O_ID2 + 1] = 1.0

        # naug chunks: [128 rows, 130 bf16] = [data | 1.0 | 0]
        nrows = np.zeros((SHARD, 2 * NW), ml_dtypes.bfloat16)
        nrows[:, 0:D] = data[lo:lo + SHARD].astype(ml_dtypes.bfloat16)
        nrows[:, D] = 1.0
        naug = np.ascontiguousarray(
            nrows.reshape(NQ, 128, 2 * NW).transpose(1, 0, 2)
        ).view(np.uint16).view(np.float32)            # [128, NQ, NW]

        in_maps.append({
            "cbw": w_all,
            "cba": cba,
            "cbb": cbb,
            "cbc": cbc,
            "dt": dtt,
            "naug": naug,
        })
    return in_maps


def run(inputs, alphas, data_batch, trace=False, trace_kwargs=None):
    nc = _get_nc()
    in_maps = _prepare_in_maps(inputs, alphas, data_batch)
    res = run_bass_kernel_spmd(
        nc, in_maps, core_ids=list(range(NCORES)),
        trace=trace, **(trace_kwargs or {}),
    )
    return res.results[0]["out"].astype(np.float32), res


def kernel(inputs, alphas, data_batch):
    out, _ = run(inputs, alphas, data_batch)
    return out
